# revision 9
# baseline (speedup 1.0000x reference)
"""Trainium2 Bass kernel for y = x @ W^T + b  (4096x4096 @ 4096x4096 + 4096).

Sharding: data-parallel over batch. Core c gets x rows [c*512:(c+1)*512];
W and b are replicated. Each core computes yT_c = W @ x_c^T + b[:, None]
(output transposed, [4096, 512]) and the host reassembles
y = concat([yT_c.T for c in cores], axis=0). No collectives.

Per-core kernel (bf16 compute, fp32 accumulate in PSUM):
  - x_c: SWDGE DMA-cast f32->bf16 into SBUF natural, transposed on the PE
    (128x128 transpose-mode blocks) into resident xT [128, 32, 512].
  - W: per 128-row slab, SWDGE DMA-cast f32->bf16 (2 chunks), PE
    transpose-mode blocks -> PSUM (GK blocks per bank), DVE eviction to
    wT [128, 32, 128]; then 32 accumulating matmuls (lhsT=wT block,
    rhs=xT block, N=512) into one PSUM bank.
  - PSUM eviction fused with bias add on ScalarE, DMA out on sync.
"""

import os
import sys

for _p in ("/opt/trn_rl_repo", "/opt/pypackages"):
    if _p not in sys.path and os.path.isdir(_p):
        sys.path.append(_p)

import numpy as np

import concourse.bass as bass
import concourse.tile as tile
from concourse import bacc, mybir
from concourse.bass_utils import run_bass_kernel_spmd

N_CORES = 8
BATCH = 4096
IN_F = 4096
OUT_F = 4096
P = 128
B = BATCH // N_CORES          # 512 batch rows per core
KT = IN_F // P                # 32 contraction tiles
OT = OUT_F // P               # 32 output-feature tiles

_F32 = mybir.dt.float32
_BF16 = mybir.dt.bfloat16

_compiled_nc = None


def _build():
    nc = bacc.Bacc("TRN2", target_bir_lowering=False, debug=False,
                   num_devices=N_CORES)

    x = nc.dram_tensor("x", [B, IN_F], _F32, kind="ExternalInput")
    w = nc.dram_tensor("weight", [OUT_F, IN_F], _F32, kind="ExternalInput")
    bias = nc.dram_tensor("bias", [OUT_F], _F32, kind="ExternalInput")
    out = nc.dram_tensor("out", [OUT_F, B], _F32, kind="ExternalOutput")

    from concourse.masks import make_identity

    GK = 4                     # k-tiles transposed per PSUM bank batch
    WH = 2                     # W cast split per slab
    BT = B // P                # 4 batch tiles
    with tile.TileContext(nc) as tc:
        with tc.tile_pool(name="const", bufs=1) as const, \
             tc.tile_pool(name="wnat", bufs=2 * WH) as wnat_pool, \
             tc.tile_pool(name="wt", bufs=3) as wt_pool, \
             tc.tile_pool(name="tpsum", bufs=3, space="PSUM") as tpsum_pool, \
             tc.tile_pool(name="psum", bufs=3, space="PSUM") as psum_pool, \
             tc.tile_pool(name="yout", bufs=3) as y_pool:

            ident = const.tile([P, P], _BF16)
            make_identity(nc, ident)
            ident32 = const.tile([32, 32], _F32)
            make_identity(nc, ident32)

            # ---- bias: one DMA to [32, 128], PE transpose -> [128, 32]
            b_nat = const.tile([32, P], _F32)
            nc.scalar.dma_start(out=b_nat[:],
                                in_=bias[:].rearrange("(a b) -> a b", b=P))
            b_ps = tpsum_pool.tile([P, 32], _F32, name="b_ps", tag="tps")
            nc.tensor.transpose(b_ps[:], b_nat[:], ident32[:])
            bias_sb = const.tile([P, OT], _F32)
            nc.vector.tensor_copy(out=bias_sb[:], in_=b_ps[:])

            # ---- x: f32 load on the (idle) HWDGE queue in BT chunks,
            # PE f32 transpose-mode, DVE cast f32->bf16 on eviction.
            ident_f32 = const.tile([P, P], _F32)
            make_identity(nc, ident_f32)
            x_nat = [const.tile([P, IN_F], _F32, name=f"xnat{bt}")
                     for bt in range(BT)]
            for bt in range(BT):
                nc.sync.dma_start(out=x_nat[bt][:],
                                  in_=x[bt * P:(bt + 1) * P, :])
            xT = const.tile([P, KT, B], _BF16)
            for kt in range(KT):
                pst = tpsum_pool.tile([P, BT, P], _F32, name=f"xps{kt}",
                                      tag="tps")
                for bt in range(BT):
                    nc.tensor.transpose(pst[:, bt, :],
                                        x_nat[bt][:, kt * P:(kt + 1) * P],
                                        ident_f32[:])
                nc.vector.tensor_copy(out=xT[:, kt, :], in_=pst[:])

            # ---- main loop over output-feature tiles
            IH = IN_F // WH
            KH = IH // P   # k-tiles per half-slab
            for ot in range(OT):
                w_nat = [wnat_pool.tile([P, IH], _BF16, tag=f"wnat{h}",
                                        name=f"wnat{h}_{ot}")
                         for h in range(WH)]
                for h in range(WH):
                    nc.gpsimd.dma_start(
                        out=w_nat[h][:],
                        in_=w[ot * P:(ot + 1) * P, h * IH:(h + 1) * IH])

                # Transpose slab on the PE (transpose-mode), GK blocks per
                # PSUM bank, DVE-evicted per bank.
                wT = wt_pool.tile([P, KT, P], _BF16)
                for g in range(KT // GK):
                    pst = tpsum_pool.tile([P, GK, P], _BF16, tag="tps",
                                          name=f"wps_{ot}_{g}")
                    for j in range(GK):
                        kt = g * GK + j
                        src = w_nat[kt // KH]
                        k0 = (kt % KH) * P
                        nc.tensor.transpose(pst[:, j, :],
                                            src[:, k0:k0 + P],
                                            ident[:])
                    nc.vector.tensor_copy(out=wT[:, g * GK:(g + 1) * GK, :],
                                          in_=pst[:])

                ps = psum_pool.tile([P, B], _F32)
                for kt in range(KT):
                    nc.tensor.matmul(ps[:], lhsT=wT[:, kt, :],
                                     rhs=xT[:, kt, :],
                                     start=(kt == 0), stop=(kt == KT - 1))

                ysb = y_pool.tile([P, B], _F32)
                nc.scalar.activation(ysb[:], ps[:],
                                     mybir.ActivationFunctionType.Identity,
                                     bias=bias_sb[:, ot:ot + 1])
                nc.sync.dma_start(out=out[ot * P:(ot + 1) * P, :], in_=ysb[:])

    nc.compile()
    return nc


def _get_nc():
    global _compiled_nc
    if _compiled_nc is None:
        _compiled_nc = _build()
    return _compiled_nc


def _run(inputs, trace=False, trace_cores=None):
    x = np.ascontiguousarray(np.asarray(inputs["x"], dtype=np.float32))
    w = np.ascontiguousarray(np.asarray(inputs["weight"], dtype=np.float32))
    b = np.ascontiguousarray(np.asarray(inputs["bias"], dtype=np.float32))

    nc = _get_nc()
    in_maps = [
        {"x": x[c * B:(c + 1) * B], "weight": w, "bias": b}
        for c in range(N_CORES)
    ]
    res = run_bass_kernel_spmd(nc, in_maps, core_ids=list(range(N_CORES)),
                               trace=trace, trace_cores=trace_cores)
    y = np.concatenate([res.results[c]["out"].T for c in range(N_CORES)], axis=0)
    return y, res


def kernel(**inputs):
    y, _ = _run(inputs)
    return y


# revision 10
# speedup vs baseline: 1.0494x; 1.0494x over previous
"""Trainium2 Bass kernel for y = x @ W^T + b  (4096x4096 @ 4096x4096 + 4096).

Sharding: data-parallel over batch. Core c gets x rows [c*512:(c+1)*512];
W and b are replicated. Each core computes yT_c = W @ x_c^T + b[:, None]
(output transposed, [4096, 512]) and the host reassembles
y = concat([yT_c.T for c in cores], axis=0). No collectives.

Per-core kernel (bf16 compute, fp32 accumulate in PSUM):
  - x_c: SWDGE DMA-cast f32->bf16 into SBUF natural, transposed on the PE
    (128x128 transpose-mode blocks) into resident xT [128, 32, 512].
  - W: per 128-row slab, SWDGE DMA-cast f32->bf16 (2 chunks), PE
    transpose-mode blocks -> PSUM (GK blocks per bank), DVE eviction to
    wT [128, 32, 128]; then 32 accumulating matmuls (lhsT=wT block,
    rhs=xT block, N=512) into one PSUM bank.
  - PSUM eviction fused with bias add on ScalarE, DMA out on sync.
"""

import os
import sys

for _p in ("/opt/trn_rl_repo", "/opt/pypackages"):
    if _p not in sys.path and os.path.isdir(_p):
        sys.path.append(_p)

import numpy as np

import concourse.bass as bass
import concourse.tile as tile
from concourse import bacc, mybir
from concourse.bass_utils import run_bass_kernel_spmd

N_CORES = 8
BATCH = 4096
IN_F = 4096
OUT_F = 4096
P = 128
B = BATCH // N_CORES          # 512 batch rows per core
KT = IN_F // P                # 32 contraction tiles
OT = OUT_F // P               # 32 output-feature tiles

_F32 = mybir.dt.float32
_BF16 = mybir.dt.bfloat16

_compiled_nc = None


def _build():
    nc = bacc.Bacc("TRN2", target_bir_lowering=False, debug=False,
                   num_devices=N_CORES)

    x = nc.dram_tensor("x", [B, IN_F], _F32, kind="ExternalInput")
    w = nc.dram_tensor("weight", [OUT_F, IN_F], _F32, kind="ExternalInput")
    bias = nc.dram_tensor("bias", [OUT_F], _F32, kind="ExternalInput")
    out = nc.dram_tensor("out", [OUT_F, B], _F32, kind="ExternalOutput")

    from concourse.masks import make_identity

    GK = 4                     # k-tiles transposed per PSUM bank batch
    WH = 2                     # W cast split per slab
    BT = B // P                # 4 batch tiles
    with tile.TileContext(nc) as tc:
        with tc.tile_pool(name="const", bufs=1) as const, \
             tc.tile_pool(name="wnat", bufs=2 * WH) as wnat_pool, \
             tc.tile_pool(name="wt", bufs=3) as wt_pool, \
             tc.tile_pool(name="tpsum", bufs=3, space="PSUM") as tpsum_pool, \
             tc.tile_pool(name="psum", bufs=3, space="PSUM") as psum_pool, \
             tc.tile_pool(name="yout", bufs=3) as y_pool:

            ident = const.tile([P, P], _BF16)
            make_identity(nc, ident)
            ident32 = const.tile([32, 32], _F32)
            make_identity(nc, ident32)

            # ---- bias: one DMA to [32, 128], PE transpose -> [128, 32]
            b_nat = const.tile([32, P], _F32)
            nc.scalar.dma_start(out=b_nat[:],
                                in_=bias[:].rearrange("(a b) -> a b", b=P))
            b_ps = tpsum_pool.tile([P, 32], _F32, name="b_ps", tag="tps")
            nc.tensor.transpose(b_ps[:], b_nat[:], ident32[:])
            bias_sb = const.tile([P, OT], _F32)
            nc.vector.tensor_copy(out=bias_sb[:], in_=b_ps[:])

            # ---- x: SWDGE DMA-cast f32->bf16 per batch tile, PE-transpose
            # (transpose-mode) into resident xT, DVE evictions per k-tile.
            x_nat = [const.tile([P, IN_F], _BF16, name=f"xnat{bt}")
                     for bt in range(BT)]
            for bt in range(BT):
                nc.gpsimd.dma_start(out=x_nat[bt][:],
                                    in_=x[bt * P:(bt + 1) * P, :])
            xT = const.tile([P, KT, B], _BF16)
            for kt in range(KT):
                pst = tpsum_pool.tile([P, BT, P], _BF16, name=f"xps{kt}",
                                      tag="tps")
                for bt in range(BT):
                    nc.tensor.transpose(pst[:, bt, :],
                                        x_nat[bt][:, kt * P:(kt + 1) * P],
                                        ident[:])
                nc.vector.tensor_copy(out=xT[:, kt, :], in_=pst[:])

            # ---- main loop over output-feature tiles
            IH = IN_F // WH
            KH = IH // P   # k-tiles per half-slab
            for ot in range(OT):
                w_nat = [wnat_pool.tile([P, IH], _BF16, tag=f"wnat{h}",
                                        name=f"wnat{h}_{ot}")
                         for h in range(WH)]
                for h in range(WH):
                    nc.gpsimd.dma_start(
                        out=w_nat[h][:],
                        in_=w[ot * P:(ot + 1) * P, h * IH:(h + 1) * IH])

                # Transpose slab on the PE (transpose-mode), GK blocks per
                # PSUM bank, DVE-evicted per bank.
                wT = wt_pool.tile([P, KT, P], _BF16)
                for g in range(KT // GK):
                    pst = tpsum_pool.tile([P, GK, P], _BF16, tag="tps",
                                          name=f"wps_{ot}_{g}")
                    for j in range(GK):
                        kt = g * GK + j
                        src = w_nat[kt // KH]
                        k0 = (kt % KH) * P
                        nc.tensor.transpose(pst[:, j, :],
                                            src[:, k0:k0 + P],
                                            ident[:])
                    nc.vector.tensor_copy(out=wT[:, g * GK:(g + 1) * GK, :],
                                          in_=pst[:])

                ps = psum_pool.tile([P, B], _F32)
                for kt in range(KT):
                    nc.tensor.matmul(ps[:], lhsT=wT[:, kt, :],
                                     rhs=xT[:, kt, :],
                                     start=(kt == 0), stop=(kt == KT - 1))

                ysb = y_pool.tile([P, B], _F32)
                nc.scalar.activation(ysb[:], ps[:],
                                     mybir.ActivationFunctionType.Identity,
                                     bias=bias_sb[:, ot:ot + 1])
                nc.sync.dma_start(out=out[ot * P:(ot + 1) * P, :], in_=ysb[:])

    nc.compile()
    return nc


def _get_nc():
    global _compiled_nc
    if _compiled_nc is None:
        _compiled_nc = _build()
    return _compiled_nc


def _run(inputs, trace=False, trace_cores=None):
    x = np.ascontiguousarray(np.asarray(inputs["x"], dtype=np.float32))
    w = np.ascontiguousarray(np.asarray(inputs["weight"], dtype=np.float32))
    b = np.ascontiguousarray(np.asarray(inputs["bias"], dtype=np.float32))

    nc = _get_nc()
    in_maps = [
        {"x": x[c * B:(c + 1) * B], "weight": w, "bias": b}
        for c in range(N_CORES)
    ]
    res = run_bass_kernel_spmd(nc, in_maps, core_ids=list(range(N_CORES)),
                               trace=trace, trace_cores=trace_cores)
    y = np.concatenate([res.results[c]["out"].T for c in range(N_CORES)], axis=0)
    return y, res


def kernel(**inputs):
    y, _ = _run(inputs)
    return y


# revision 11
# speedup vs baseline: 1.2558x; 1.1966x over previous
"""Trainium2 Bass kernel for y = x @ W^T + b  (4096x4096 @ 4096x4096 + 4096).

Sharding: data-parallel over batch. Core c gets x rows [c*512:(c+1)*512];
W and b are replicated. Each core computes yT_c = W @ x_c^T + b[:, None]
(output transposed, [4096, 512]) and the host reassembles
y = concat([yT_c.T for c in cores], axis=0). No collectives.

Per-core kernel (bf16 compute, fp32 accumulate in PSUM):
  - x_c: SWDGE DMA-cast f32->bf16 into SBUF natural, transposed on the PE
    (128x128 transpose-mode blocks) into resident xT [128, 32, 512].
  - W: per 128-row slab, SWDGE DMA-cast f32->bf16 (2 chunks), PE
    transpose-mode blocks -> PSUM (GK blocks per bank), DVE eviction to
    wT [128, 32, 128]; then 32 accumulating matmuls (lhsT=wT block,
    rhs=xT block, N=512) into one PSUM bank.
  - PSUM eviction fused with bias add on ScalarE, DMA out on sync.
"""

import os
import sys

for _p in ("/opt/trn_rl_repo", "/opt/pypackages"):
    if _p not in sys.path and os.path.isdir(_p):
        sys.path.append(_p)

import numpy as np

import concourse.bass as bass
import concourse.tile as tile
from concourse import bacc, mybir
from concourse.bass_utils import run_bass_kernel_spmd

N_CORES = 8
BATCH = 4096
IN_F = 4096
OUT_F = 4096
P = 128
B = BATCH // N_CORES          # 512 batch rows per core
KT = IN_F // P                # 32 contraction tiles
OT = OUT_F // P               # 32 output-feature tiles

_F32 = mybir.dt.float32
_BF16 = mybir.dt.bfloat16

_compiled_nc = None


def _build():
    nc = bacc.Bacc("TRN2", target_bir_lowering=False, debug=False,
                   num_devices=N_CORES)

    x = nc.dram_tensor("x", [B, IN_F], _F32, kind="ExternalInput")
    w = nc.dram_tensor("weight", [OUT_F, IN_F], _F32, kind="ExternalInput")
    bias = nc.dram_tensor("bias", [OUT_F], _F32, kind="ExternalInput")
    out = nc.dram_tensor("out", [OUT_F, B], _F32, kind="ExternalOutput")

    from concourse.masks import make_identity

    GK = 4                     # k-tiles transposed per PSUM bank batch
    WH = 2                     # W cast split per slab
    BT = B // P                # 4 batch tiles
    with tile.TileContext(nc) as tc:
        with tc.tile_pool(name="const", bufs=1) as const, \
             tc.tile_pool(name="wnat", bufs=2 * WH) as wnat_pool, \
             tc.tile_pool(name="wt", bufs=3) as wt_pool, \
             tc.tile_pool(name="tpsum", bufs=3, space="PSUM") as tpsum_pool, \
             tc.tile_pool(name="psum", bufs=3, space="PSUM") as psum_pool, \
             tc.tile_pool(name="yout", bufs=3) as y_pool:

            ident = const.tile([P, P], _BF16)
            make_identity(nc, ident)
            ident32 = const.tile([32, 32], _F32)
            make_identity(nc, ident32)

            # ---- bias: one DMA to [32, 128], PE transpose -> [128, 32]
            b_nat = const.tile([32, P], _F32)
            nc.scalar.dma_start(out=b_nat[:],
                                in_=bias[:].rearrange("(a b) -> a b", b=P))
            b_ps = tpsum_pool.tile([P, 32], _F32, name="b_ps", tag="tps")
            nc.tensor.transpose(b_ps[:], b_nat[:], ident32[:])
            bias_sb = const.tile([P, OT], _F32)
            nc.vector.tensor_copy(out=bias_sb[:], in_=b_ps[:])

            # ---- x: DMA-cast to SBUF natural, PE-transpose to xT (resident)
            x_nat = const.tile([P, BT, IN_F], _BF16)
            nc.gpsimd.dma_start(
                out=x_nat[:],
                in_=x[:, :].rearrange("(bt p) i -> p bt i", p=P))
            xT = const.tile([P, KT, B], _BF16)
            for kt in range(KT):
                pst = tpsum_pool.tile([P, BT, P], _BF16, name=f"xps{kt}",
                                      tag="tps")
                for bt in range(BT):
                    nc.tensor.transpose(pst[:, bt, :],
                                        x_nat[:, bt, kt * P:(kt + 1) * P],
                                        ident[:])
                nc.vector.tensor_copy(out=xT[:, kt, :], in_=pst[:])

            # ---- main loop over output-feature tiles
            IH = IN_F // WH
            KH = IH // P   # k-tiles per half-slab
            for ot in range(OT):
                w_nat = [wnat_pool.tile([P, IH], _BF16, tag=f"wnat{h}",
                                        name=f"wnat{h}_{ot}")
                         for h in range(WH)]
                for h in range(WH):
                    nc.gpsimd.dma_start(
                        out=w_nat[h][:],
                        in_=w[ot * P:(ot + 1) * P, h * IH:(h + 1) * IH])

                # Transpose slab on the PE (transpose-mode), GK blocks per
                # PSUM bank, DVE-evicted per bank.
                wT = wt_pool.tile([P, KT, P], _BF16)
                for g in range(KT // GK):
                    pst = tpsum_pool.tile([P, GK, P], _BF16, tag="tps",
                                          name=f"wps_{ot}_{g}")
                    for j in range(GK):
                        kt = g * GK + j
                        src = w_nat[kt // KH]
                        k0 = (kt % KH) * P
                        nc.tensor.transpose(pst[:, j, :],
                                            src[:, k0:k0 + P],
                                            ident[:])
                    nc.vector.tensor_copy(out=wT[:, g * GK:(g + 1) * GK, :],
                                          in_=pst[:])

                ps = psum_pool.tile([P, B], _F32)
                for kt in range(KT):
                    nc.tensor.matmul(ps[:], lhsT=wT[:, kt, :],
                                     rhs=xT[:, kt, :],
                                     start=(kt == 0), stop=(kt == KT - 1))

                ysb = y_pool.tile([P, B], _F32)
                nc.scalar.activation(ysb[:], ps[:],
                                     mybir.ActivationFunctionType.Identity,
                                     bias=bias_sb[:, ot:ot + 1])
                nc.sync.dma_start(out=out[ot * P:(ot + 1) * P, :], in_=ysb[:])

    nc.compile()
    return nc


def _get_nc():
    global _compiled_nc
    if _compiled_nc is None:
        _compiled_nc = _build()
    return _compiled_nc


def _run(inputs, trace=False, trace_cores=None):
    x = np.ascontiguousarray(np.asarray(inputs["x"], dtype=np.float32))
    w = np.ascontiguousarray(np.asarray(inputs["weight"], dtype=np.float32))
    b = np.ascontiguousarray(np.asarray(inputs["bias"], dtype=np.float32))

    nc = _get_nc()
    in_maps = [
        {"x": x[c * B:(c + 1) * B], "weight": w, "bias": b}
        for c in range(N_CORES)
    ]
    res = run_bass_kernel_spmd(nc, in_maps, core_ids=list(range(N_CORES)),
                               trace=trace, trace_cores=trace_cores)
    y = np.concatenate([res.results[c]["out"].T for c in range(N_CORES)], axis=0)
    return y, res


def kernel(**inputs):
    y, _ = _run(inputs)
    return y
